# revision 19
# baseline (speedup 1.0000x reference)
"""BERT-embedding kernel for 8 Trainium2 NeuronCores (Bass/Tile).

out[b,s,:] = concat( input[b,s,:] @ W.T + b_vec,  PE[doy[b,s], :] )
with PE the standard sinusoidal table (d_model=256, max_len=366).

Strategy (data-parallel over batch, 8 cores):
  - The harness checks kernel()'s returned float32 array at rel-err < 2e-2,
    so the device-resident output is bf16 (cast to f32 on host).  That
    halves the dominant HBM write traffic (33.5 -> 16.8 MB/core) and moves
    the roofline from ~95us to ~50us; elementwise compute then paces.
  - obs half: bf16 TensorE matmul; two token tiles packed per matmul with a
    block-diagonal stationary operand (K=2*11=22, N=512 = one PSUM bank);
    two matmuls share a 2-bank PSUM tile evacuated by a single copy.
  - PE half, computed in TURNS to minimize DVE work:
      t = doy * (div/2pi)            one tensor_tensor    [128 cols]
      b = (t + 0.5) mod 1.0          one tensor_scalar    [128 cols]
    then ACT's free affine does the rest:
      sin col = Sin( 2pi*b - pi )              ( = sin(2pi*t) )
      a = Abs( b - 0.5 )                       ( = |t - round(t)| )
      cos col = Sin( -2pi*a + pi/2 )           ( = cos(2pi*t) )
    The Sin spline is valid on [-pi, pi]; all arguments stay inside.
  - inputs merged into two tensors (aux = doyT|div2pi table,
    lt_all = rhs|packed-lhs) so only 3 input DMAs are issued.
"""
import numpy as np

# ---------------- problem constants (hardcoded per contract) ----------------
B, S, F, D = 1024, 128, 10, 256
MAX_LEN = 366
N_CORES = 8
BPC = B // N_CORES          # batches per core
TOK = BPC * S               # tokens per core = 16384
P = 128                     # tokens per tile (SBUF partitions)
G = TOK // P                # 128 tiles per core
GROUP_PLAN = [2, 2, 4, 8] + [8] * 12 + [6, 4, 4, 2]
assert sum(GROUP_PLAN) == G
K = F + 1                   # contraction dim incl. bias row
K2 = 2 * K                  # packed two-tile contraction dim

PI = float(np.float32(np.pi))
HALF_PI = float(np.float32(np.pi / 2))
TWO_PI = float(np.float32(2 * np.pi))

REDUCE_MODE = "magic"       # "mod" | "magic"; DVE ISA has no mod -> magic
MAGIC = 12582912.0          # 1.5 * 2**23 (magic-rounding fallback)
R = 68                      # cols needing reduction in "magic" mode

# of the 2-bank-chunk PSUM->SBUF copies, route this many (num, den) to ACT
ACT_COPY_RATIO = (4, 9)

HEAD = 8                    # lhs pairs in piece 1 (covers groups 0..3)
RHS_COLS = 2 * D            # rhs block-diag packed at cols 0:512 of lt_all

_CACHE = {}


def _copy_on_act(chunk_idx):
    num, den = ACT_COPY_RATIO
    return (chunk_idx * num) % den < num


def _build_nc():
    import concourse.bacc as bacc
    import concourse.tile as tile
    import concourse.mybir as mybir

    F32 = mybir.dt.float32
    BF16 = mybir.dt.bfloat16
    AOT = mybir.AluOpType
    ACT = mybir.ActivationFunctionType

    nc = bacc.Bacc("TRN2", target_bir_lowering=False, debug=False,
                   num_devices=N_CORES)
    aux_d = nc.dram_tensor("aux", [P, 256], F32, kind="ExternalInput")
    lt_d = nc.dram_tensor(
        "ltall", [K2, RHS_COLS + (G // 2) * P], BF16, kind="ExternalInput"
    )
    out_d = nc.dram_tensor("out", [TOK, 2 * D], BF16, kind="ExternalOutput")

    # out rows viewed as (t, p): row = t*P + p
    outv = out_d[:].rearrange("(t p) c -> p t c", p=P)
    CUT = RHS_COLS + HEAD * P

    with tile.TileContext(nc) as tc:
        with (
            tc.tile_pool(name="const", bufs=1) as cpool,
            tc.tile_pool(name="angp", bufs=4) as angp,
            tc.tile_pool(name="outp", bufs=6) as outp,
            tc.tile_pool(name="psum", bufs=3, space="PSUM") as psump,
        ):
            aux_sb = cpool.tile([P, 256], F32)
            nc.sync.dma_start(aux_sb[:], aux_d[:])
            lt_sb = cpool.tile([K2, RHS_COLS + (G // 2) * P], BF16)
            nc.sync.dma_start(lt_sb[:, 0:CUT], lt_d[:, 0:CUT])
            halfpi = cpool.tile([P, 1], F32)
            nc.vector.memset(halfpi[:], HALF_PI)
            minuspi = cpool.tile([P, 1], F32)
            nc.vector.memset(minuspi[:], -PI)
            minushalf = cpool.tile([P, 1], F32)
            nc.vector.memset(minushalf[:], -0.5)
            # warm the trig table during the preamble (Sin/Abs/Copy share it)
            warm = cpool.tile([P, 1], F32)
            nc.scalar.activation(warm[:], halfpi[:], ACT.Sin)
            nc.scalar.activation(warm[:], halfpi[:], ACT.Abs)
            # rest of lhs resident
            nc.sync.dma_start(lt_sb[:, CUT:], lt_d[:, CUT:])
            rhs_ap = lt_sb[:, 0:RHS_COLS]

            t0 = 0
            chunk0 = 0
            for tpg in GROUP_PLAN:
                npair = tpg // 2
                p0 = t0 // 2

                og = outp.tile([P, tpg, 2 * D], BF16, tag="og")
                tg = angp.tile([P, tpg, 128], F32, tag="tg")

                # obs half: two matmuls share one 2-bank PSUM tile; single
                # copy moves 4 token-tiles of obs data and casts to bf16
                for c in range(0, npair, 2):
                    nj = min(2, npair - c)
                    ps = psump.tile([P, 2, 512], F32, tag="ps")
                    for j in range(nj):
                        pair = p0 + c + j
                        nc.tensor.matmul(
                            ps[:, j, :],
                            lt_sb[:, RHS_COLS + pair * P:
                                  RHS_COLS + (pair + 1) * P],
                            rhs_ap,
                        )
                    src = ps[:, 0:nj, :].rearrange(
                        "p a (t c) -> p (a t) c", t=2
                    )
                    dst = og[:, 2 * c:2 * c + 2 * nj, 0:D]
                    if _copy_on_act(chunk0):
                        nc.scalar.copy(dst, src)
                    else:
                        nc.vector.tensor_copy(out=dst, in_=src)
                    chunk0 += 1

                # t[p,tt,i] = doy[p, t0+tt] * div2pi[i]   (turns)
                div_b = (
                    aux_sb[:, 128:256].rearrange("p i -> p () i")
                    .to_broadcast([P, tpg, 128])
                )
                doy_b = (
                    aux_sb[:, t0:t0 + tpg]
                    .rearrange("p t -> p t ()")
                    .to_broadcast([P, tpg, 128])
                )
                nc.vector.tensor_tensor(out=tg[:], in0=div_b, in1=doy_b,
                                        op=AOT.mult)

                if REDUCE_MODE == "mod":
                    # b = (t + 0.5) mod 1.0  ->  b - 0.5 = t - round(t)
                    nc.vector.tensor_scalar(
                        out=tg[:], in0=tg[:], scalar1=0.5, scalar2=1.0,
                        op0=AOT.add, op1=AOT.mod,
                    )
                    ay = angp.tile([P, tpg, 128], F32, tag="ay")
                    nc.scalar.activation(ay[:], tg[:], ACT.Abs,
                                         bias=minushalf[:])
                    nc.scalar.activation(og[:, :, D::2], tg[:], ACT.Sin,
                                         scale=TWO_PI, bias=minuspi[:])
                    nc.scalar.activation(
                        og[:, :, D + 1::2], ay[:], ACT.Sin,
                        scale=-TWO_PI, bias=halfpi[:],
                    )
                else:
                    # magic-number reduction (fallback): uc = round(t)
                    uc = angp.tile([P, tpg, R], F32, tag="uc")
                    nc.vector.tensor_scalar(
                        out=uc[:], in0=tg[:, :, 0:R], scalar1=MAGIC,
                        scalar2=MAGIC, op0=AOT.add, op1=AOT.subtract,
                    )
                    nc.vector.tensor_tensor(
                        out=tg[:, :, 0:R], in0=tg[:, :, 0:R],
                        in1=uc[:], op=AOT.subtract,
                    )
                    # device stores sin block at cols 256:384 and cos block
                    # at 384:512 (contiguous writes are ~20% faster on ACT
                    # than stride-2); the host interleaves after readback
                    nc.scalar.activation(og[:, :, D:D + 128], tg[:], ACT.Sin,
                                         scale=TWO_PI)
                    # cos(2pi*t) = sin(pi/2 - 2pi*t_red); uses the Sin
                    # spline up to 3pi/2 (t_red in [-1/2,1/2])
                    nc.scalar.activation(
                        og[:, :, D + 128:2 * D], tg[:], ACT.Sin,
                        scale=-TWO_PI, bias=halfpi[:],
                    )

                if t0 < 8:
                    nc.sync.dma_start(
                        outv[:, t0:t0 + tpg, 0:D], og[:, :, 0:D]
                    )
                    nc.sync.dma_start(
                        outv[:, t0:t0 + tpg, D:2 * D], og[:, :, D:2 * D]
                    )
                else:
                    nc.sync.dma_start(outv[:, t0:t0 + tpg, :], og[:])
                t0 += tpg
    nc.compile()
    return nc


def _host_prep(input_sequence, doy_sequence, W, b):
    import ml_dtypes
    bf16 = ml_dtypes.bfloat16
    x = np.ascontiguousarray(np.asarray(input_sequence, dtype=np.float32))
    doy = np.asarray(doy_sequence)
    Wf = np.asarray(W, dtype=np.float32)
    bf = np.asarray(b, dtype=np.float32)

    # block-diagonal rhs [2K, 2D]
    rhs = np.zeros((K2, 2 * D), dtype=np.float32)
    rhs[:F, :D] = Wf.T
    rhs[F, :D] = bf
    rhs[K:K + F, D:] = Wf.T
    rhs[K + F, D:] = bf

    div2 = (
        np.exp(np.arange(0, D, 2, dtype=np.float32)
               * np.float32(-np.log(10000.0) / D))
        / np.float32(2 * np.pi)
    ).astype(np.float32)

    xs = x.reshape(N_CORES, TOK, F)
    ds = doy.reshape(N_CORES, TOK).astype(np.float32)

    in_maps = []
    for c in range(N_CORES):
        # packed lhs: [2K, TOK/2]; tiles interleaved pairwise
        xt = xs[c].reshape(G, P, F)          # [tile, p, f]
        lhs = np.zeros((K2, TOK // 2), dtype=np.float32)
        xt_even = xt[0::2]                   # [G/2, P, F]
        xt_odd = xt[1::2]
        lhs[:F] = xt_even.transpose(2, 0, 1).reshape(F, TOK // 2)
        lhs[F] = 1.0
        lhs[K:K + F] = xt_odd.transpose(2, 0, 1).reshape(F, TOK // 2)
        lhs[K + F] = 1.0
        ltall = np.concatenate([rhs, lhs], axis=1).astype(bf16)
        doyT = np.ascontiguousarray(ds[c].reshape(G, P).T)
        aux = np.concatenate(
            [doyT, np.broadcast_to(div2, (P, D // 2))], axis=1
        ).astype(np.float32)
        in_maps.append({"ltall": ltall, "aux": aux})
    return in_maps


def _get_nc():
    if "nc" not in _CACHE:
        _CACHE["nc"] = _build_nc()
    return _CACHE["nc"]


def kernel(input_sequence, doy_sequence, W, b, _trace=False, _trace_kwargs=None):
    from concourse.bass_utils import run_bass_kernel_spmd

    nc = _get_nc()
    in_maps = _host_prep(input_sequence, doy_sequence, W, b)
    kw = {}
    if _trace:
        kw.update(trace=True, **(_trace_kwargs or {}))
    res = run_bass_kernel_spmd(nc, in_maps, core_ids=list(range(N_CORES)), **kw)
    dev = np.concatenate(
        [np.asarray(res.results[c]["out"]).astype(np.float32)
         for c in range(N_CORES)], axis=0
    )
    # device stores the PE half as [sin x128 | cos x128]; interleave here
    out = np.empty_like(dev)
    out[:, 0:D] = dev[:, 0:D]
    out[:, D::2] = dev[:, D:D + 128]
    out[:, D + 1::2] = dev[:, D + 128:2 * D]
    out = out.reshape(B, S, 2 * D)
    if _trace:
        _CACHE["last_results"] = res
    return out


# revision 23
# speedup vs baseline: 1.0440x; 1.0440x over previous
"""BERT-embedding kernel for 8 Trainium2 NeuronCores (Bass/Tile).

out[b,s,:] = concat( input[b,s,:] @ W.T + b_vec,  PE[doy[b,s], :] )
with PE the standard sinusoidal table (d_model=256, max_len=366).

Strategy (data-parallel over batch, 8 cores):
  - The harness checks kernel()'s returned float32 array at rel-err < 2e-2,
    so the device-resident output is bf16 (cast to f32 on host).  That
    halves the dominant HBM write traffic (33.5 -> 16.8 MB/core) and moves
    the roofline from ~95us to ~50us; elementwise compute then paces.
  - obs half: bf16 TensorE matmul; two token tiles packed per matmul with a
    block-diagonal stationary operand (K=2*11=22, N=512 = one PSUM bank);
    two matmuls share a 2-bank PSUM tile evacuated by a single copy.
  - PE half, computed in TURNS to minimize DVE work:
      t = doy * (div/2pi)            one tensor_tensor    [128 cols]
      b = (t + 0.5) mod 1.0          one tensor_scalar    [128 cols]
    then ACT's free affine does the rest:
      sin col = Sin( 2pi*b - pi )              ( = sin(2pi*t) )
      a = Abs( b - 0.5 )                       ( = |t - round(t)| )
      cos col = Sin( -2pi*a + pi/2 )           ( = cos(2pi*t) )
    The Sin spline is valid on [-pi, pi]; all arguments stay inside.
  - inputs merged into two tensors (aux = doyT|div2pi table,
    lt_all = rhs|packed-lhs) so only 3 input DMAs are issued.
"""
import numpy as np

# ---------------- problem constants (hardcoded per contract) ----------------
B, S, F, D = 1024, 128, 10, 256
MAX_LEN = 366
N_CORES = 8
BPC = B // N_CORES          # batches per core
TOK = BPC * S               # tokens per core = 16384
P = 128                     # tokens per tile (SBUF partitions)
G = TOK // P                # 128 tiles per core
GROUP_PLAN = [2, 2, 4, 8] + [8] * 13 + [4, 2, 2]
assert sum(GROUP_PLAN) == G
K = F + 1                   # contraction dim incl. bias row
K2 = 2 * K                  # packed two-tile contraction dim

PI = float(np.float32(np.pi))
HALF_PI = float(np.float32(np.pi / 2))
TWO_PI = float(np.float32(2 * np.pi))

REDUCE_MODE = "magic"       # "mod" | "magic"; DVE ISA has no mod -> magic
MAGIC = 12582912.0          # 1.5 * 2**23 (magic-rounding fallback)
R = 68                      # cols needing reduction in "magic" mode

# of the 2-bank-chunk PSUM->SBUF copies, route this many (num, den) to ACT
ACT_COPY_RATIO = (1, 2)

HEAD = 8                    # lhs pairs in piece 1 (covers groups 0..3)
RHS_COLS = 2 * D            # rhs block-diag packed at cols 0:512 of lt_all

_CACHE = {}


def _copy_on_act(chunk_idx):
    # ramp chunks all on ACT: it idles there (first sins wait on DVE's
    # angle chains) while DVE is busy producing tg for the early groups
    if chunk_idx < 5:
        return True
    num, den = ACT_COPY_RATIO
    return (chunk_idx * num) % den < num


def _build_nc():
    import concourse.bacc as bacc
    import concourse.tile as tile
    import concourse.mybir as mybir

    F32 = mybir.dt.float32
    BF16 = mybir.dt.bfloat16
    AOT = mybir.AluOpType
    ACT = mybir.ActivationFunctionType

    nc = bacc.Bacc("TRN2", target_bir_lowering=False, debug=False,
                   num_devices=N_CORES)
    aux_d = nc.dram_tensor("aux", [P, 256], F32, kind="ExternalInput")
    lt_d = nc.dram_tensor(
        "ltall", [K2, RHS_COLS + (G // 2) * P], BF16, kind="ExternalInput"
    )
    out_d = nc.dram_tensor("out", [TOK, 2 * D], BF16, kind="ExternalOutput")

    # out rows viewed as (t, p): row = t*P + p
    outv = out_d[:].rearrange("(t p) c -> p t c", p=P)
    CUT = RHS_COLS + HEAD * P

    with tile.TileContext(nc) as tc:
        with (
            tc.tile_pool(name="const", bufs=1) as cpool,
            tc.tile_pool(name="angp", bufs=4) as angp,
            tc.tile_pool(name="outp", bufs=6) as outp,
            tc.tile_pool(name="psum", bufs=3, space="PSUM") as psump,
        ):
            aux_sb = cpool.tile([P, 256], F32)
            nc.sync.dma_start(aux_sb[:], aux_d[:])
            lt_sb = cpool.tile([K2, RHS_COLS + (G // 2) * P], BF16)
            nc.sync.dma_start(lt_sb[:, 0:CUT], lt_d[:, 0:CUT])
            halfpi = cpool.tile([P, 1], F32)
            nc.vector.memset(halfpi[:], HALF_PI)
            minuspi = cpool.tile([P, 1], F32)
            nc.vector.memset(minuspi[:], -PI)
            minushalf = cpool.tile([P, 1], F32)
            nc.vector.memset(minushalf[:], -0.5)
            # warm the trig table during the preamble (Sin/Copy share a set)
            warm = cpool.tile([P, 1], F32)
            nc.scalar.activation(warm[:], halfpi[:], ACT.Sin)
            # rest of lhs resident
            nc.sync.dma_start(lt_sb[:, CUT:], lt_d[:, CUT:])
            rhs_ap = lt_sb[:, 0:RHS_COLS]

            t0 = 0
            chunk0 = 0
            for tpg in GROUP_PLAN:
                npair = tpg // 2
                p0 = t0 // 2

                og = outp.tile([P, tpg, 2 * D], BF16, tag="og")
                tg = angp.tile([P, tpg, 128], F32, tag="tg")

                # obs half: two matmuls share one 2-bank PSUM tile; single
                # copy moves 4 token-tiles of obs data and casts to bf16
                for c in range(0, npair, 2):
                    nj = min(2, npair - c)
                    ps = psump.tile([P, 2, 512], F32, tag="ps")
                    for j in range(nj):
                        pair = p0 + c + j
                        nc.tensor.matmul(
                            ps[:, j, :],
                            lt_sb[:, RHS_COLS + pair * P:
                                  RHS_COLS + (pair + 1) * P],
                            rhs_ap,
                        )
                    src = ps[:, 0:nj, :].rearrange(
                        "p a (t c) -> p (a t) c", t=2
                    )
                    dst = og[:, 2 * c:2 * c + 2 * nj, 0:D]
                    if _copy_on_act(chunk0):
                        nc.scalar.copy(dst, src)
                    else:
                        nc.vector.tensor_copy(out=dst, in_=src)
                    chunk0 += 1

                # t[p,tt,i] = doy[p, t0+tt] * div2pi[i]   (turns)
                div_b = (
                    aux_sb[:, 128:256].rearrange("p i -> p () i")
                    .to_broadcast([P, tpg, 128])
                )
                doy_b = (
                    aux_sb[:, t0:t0 + tpg]
                    .rearrange("p t -> p t ()")
                    .to_broadcast([P, tpg, 128])
                )
                nc.vector.tensor_tensor(out=tg[:], in0=div_b, in1=doy_b,
                                        op=AOT.mult)

                if REDUCE_MODE == "mod":
                    # b = (t + 0.5) mod 1.0  ->  b - 0.5 = t - round(t)
                    nc.vector.tensor_scalar(
                        out=tg[:], in0=tg[:], scalar1=0.5, scalar2=1.0,
                        op0=AOT.add, op1=AOT.mod,
                    )
                    ay = angp.tile([P, tpg, 128], F32, tag="ay")
                    nc.scalar.activation(ay[:], tg[:], ACT.Abs,
                                         bias=minushalf[:])
                    nc.scalar.activation(og[:, :, D::2], tg[:], ACT.Sin,
                                         scale=TWO_PI, bias=minuspi[:])
                    nc.scalar.activation(
                        og[:, :, D + 1::2], ay[:], ACT.Sin,
                        scale=-TWO_PI, bias=halfpi[:],
                    )
                else:
                    # magic-number reduction (fallback): uc = round(t)
                    uc = angp.tile([P, tpg, R], F32, tag="uc")
                    nc.vector.tensor_scalar(
                        out=uc[:], in0=tg[:, :, 0:R], scalar1=MAGIC,
                        scalar2=MAGIC, op0=AOT.add, op1=AOT.subtract,
                    )
                    nc.vector.tensor_tensor(
                        out=tg[:, :, 0:R], in0=tg[:, :, 0:R],
                        in1=uc[:], op=AOT.subtract,
                    )
                    # device stores sin block at cols 256:384 and cos block
                    # at 384:512 (contiguous writes are ~20% faster on ACT
                    # than stride-2); the host interleaves after readback
                    nc.scalar.activation(og[:, :, D:D + 128], tg[:], ACT.Sin,
                                         scale=TWO_PI)
                    # cos(2pi*t) = sin(pi/2 - 2pi*t_red); uses the Sin
                    # spline up to 3pi/2 (t_red in [-1/2,1/2])
                    nc.scalar.activation(
                        og[:, :, D + 128:2 * D], tg[:], ACT.Sin,
                        scale=-TWO_PI, bias=halfpi[:],
                    )

                if t0 < 8:
                    nc.sync.dma_start(
                        outv[:, t0:t0 + tpg, 0:D], og[:, :, 0:D]
                    )
                    nc.sync.dma_start(
                        outv[:, t0:t0 + tpg, D:2 * D], og[:, :, D:2 * D]
                    )
                elif tpg == 8:
                    # half-group DMAs halve the queued backlog that must
                    # drain after the last compute finishes
                    h = tpg // 2
                    nc.sync.dma_start(
                        outv[:, t0:t0 + h, :], og[:, 0:h, :]
                    )
                    nc.sync.dma_start(
                        outv[:, t0 + h:t0 + tpg, :], og[:, h:tpg, :]
                    )
                else:
                    nc.sync.dma_start(outv[:, t0:t0 + tpg, :], og[:])
                t0 += tpg
    nc.compile()
    return nc


def _host_prep(input_sequence, doy_sequence, W, b):
    import ml_dtypes
    bf16 = ml_dtypes.bfloat16
    x = np.ascontiguousarray(np.asarray(input_sequence, dtype=np.float32))
    doy = np.asarray(doy_sequence)
    Wf = np.asarray(W, dtype=np.float32)
    bf = np.asarray(b, dtype=np.float32)

    # block-diagonal rhs [2K, 2D]
    rhs = np.zeros((K2, 2 * D), dtype=np.float32)
    rhs[:F, :D] = Wf.T
    rhs[F, :D] = bf
    rhs[K:K + F, D:] = Wf.T
    rhs[K + F, D:] = bf

    div2 = (
        np.exp(np.arange(0, D, 2, dtype=np.float32)
               * np.float32(-np.log(10000.0) / D))
        / np.float32(2 * np.pi)
    ).astype(np.float32)

    xs = x.reshape(N_CORES, TOK, F)
    ds = doy.reshape(N_CORES, TOK).astype(np.float32)

    in_maps = []
    for c in range(N_CORES):
        # packed lhs: [2K, TOK/2]; tiles interleaved pairwise
        xt = xs[c].reshape(G, P, F)          # [tile, p, f]
        lhs = np.zeros((K2, TOK // 2), dtype=np.float32)
        xt_even = xt[0::2]                   # [G/2, P, F]
        xt_odd = xt[1::2]
        lhs[:F] = xt_even.transpose(2, 0, 1).reshape(F, TOK // 2)
        lhs[F] = 1.0
        lhs[K:K + F] = xt_odd.transpose(2, 0, 1).reshape(F, TOK // 2)
        lhs[K + F] = 1.0
        ltall = np.concatenate([rhs, lhs], axis=1).astype(bf16)
        doyT = np.ascontiguousarray(ds[c].reshape(G, P).T)
        aux = np.concatenate(
            [doyT, np.broadcast_to(div2, (P, D // 2))], axis=1
        ).astype(np.float32)
        in_maps.append({"ltall": ltall, "aux": aux})
    return in_maps


def _get_nc():
    if "nc" not in _CACHE:
        _CACHE["nc"] = _build_nc()
    return _CACHE["nc"]


def kernel(input_sequence, doy_sequence, W, b, _trace=False, _trace_kwargs=None):
    from concourse.bass_utils import run_bass_kernel_spmd

    nc = _get_nc()
    in_maps = _host_prep(input_sequence, doy_sequence, W, b)
    kw = {}
    if _trace:
        kw.update(trace=True, **(_trace_kwargs or {}))
    res = run_bass_kernel_spmd(nc, in_maps, core_ids=list(range(N_CORES)), **kw)
    dev = np.concatenate(
        [np.asarray(res.results[c]["out"]).astype(np.float32)
         for c in range(N_CORES)], axis=0
    )
    # device stores the PE half as [sin x128 | cos x128]; interleave here
    out = np.empty_like(dev)
    out[:, 0:D] = dev[:, 0:D]
    out[:, D::2] = dev[:, D:D + 128]
    out[:, D + 1::2] = dev[:, D + 128:2 * D]
    out = out.reshape(B, S, 2 * D)
    if _trace:
        _CACHE["last_results"] = res
    return out
